# revision 42
# baseline (speedup 1.0000x reference)
"""Self-attention layer (softmax(X @ X^T) @ X) on 8 Trainium2 NeuronCores.

Data-parallel over batch: each of the 8 cores computes one batch element's
full attention for X of shape [2048, 512].

Per-core algorithm. Scores are computed TRANSPOSED (T[j, i] = <x_i, x_j>,
key index j on partitions) so the exponentiated tile is already in the
layout the PV matmul needs as its stationary operand. QK matmuls run in
fp8-e4m3 DoubleRow (2 MACs/cell/cycle, K=256 per matmul).

Softmax split: with stabilizer c_i = |x_i|^2 = s_ii, the diagonal of
E = exp(s - c) is exactly 1, so E = I + E_off.  E_off (diag zeroed by a
(1-I) mask) is stored in fp8 and the PV contraction runs in fp8 DoubleRow
(2x PE throughput); the identity term rides as ONE extra bf16 matmul
(lhsT = I, rhs = X_bf) accumulating into the same PSUM bank, so x_i enters
the output at bf16 precision, untouched by fp8 quantization.  l = 1 + sum_j
E_off also rides the PV weights as N=1 matmuls (the "1" from the identity
matmul's ones-column ride).  Output o_i = psum_i / l_i via the scalar
engine's activation Copy with per-partition scale = 1/l.

QK tiles are produced in PAIRS into a 2-bank psum tile [128, 1024]: the
-c bias add runs as one broadcast DVE op and the exp as ONE scalar
activation per pair (halves the per-tile fixed overhead on the drain
path, which otherwise stalls the QK matmuls on psum-bank recycling).

Scheduling notes (the engine queues are strictly in-order, so emission
order == execution order per engine):
  - warmup + boundary dummy matmuls keep the PE HAM clock gate at 8/8
    (an idle gap > ~3.4us halves the PE clock for ~10us);
  - transposes read the f32 input directly (full-rate fp32 transpose
    mode), so they never wait on a cast;
  - X_bf casts are emitted after exps: the qk drain chain stays ahead;
  - in the main loop PV steps LEAD the QK stream (PV operands are a
    chunk old), leading by two at chunk boundaries, and each block
    consumes key pairs in order [2,3,4,5],[0,1,6,7] so the freshest
    exp output is needed last.
"""

import os
import numpy as np

B, S, D = 8, 2048, 512
P = 128
NI = S // P  # 16 row blocks
NK = D // P  # 4 d-tiles
JC = 512     # query column chunk (one psum bank)
NC = S // JC  # 4 chunks
NSUB = JC // P  # 4 i-tiles per chunk
NPAIR = NI // 2  # 8 key-block pairs for DoubleRow PV

_CACHE = {}


def _build_nc():
    from contextlib import ExitStack

    import concourse.bacc as bacc
    import concourse.mybir as mybir
    import concourse.tile as tile
    from concourse import masks

    f32 = mybir.dt.float32
    bf16 = mybir.dt.bfloat16
    fp8 = mybir.dt.float8e4
    AF = mybir.ActivationFunctionType
    DR = mybir.MatmulPerfMode.DoubleRow

    nc = bacc.Bacc("TRN2", target_bir_lowering=False, debug=False, num_devices=B)
    inp = nc.dram_tensor("inputs", [S, D], f32, kind="ExternalInput").ap()
    out = nc.dram_tensor("out", [S, D], f32, kind="ExternalOutput").ap()

    with tile.TileContext(nc) as tc, ExitStack() as ctx:
        const_pool = ctx.enter_context(tc.tile_pool(name="const", bufs=1))
        persist = ctx.enter_context(tc.tile_pool(name="persist", bufs=1))
        stat_pool = ctx.enter_context(tc.tile_pool(name="stat", bufs=3))
        osb_pool = ctx.enter_context(tc.tile_pool(name="osb", bufs=3))
        # PSUM budget (8 banks): qk 2x2 + tr 2 + pv 2; the l accumulator
        # reuses a tr bank after the ramp (negc uses the qk pool, so the tr
        # pool only ever holds transpose tiles and then l2).
        qk_psum = ctx.enter_context(tc.tile_pool(name="qk_psum", bufs=2, space="PSUM"))
        tr_psum = ctx.enter_context(tc.tile_pool(name="tr_psum", bufs=2, space="PSUM"))
        pv_psum = ctx.enter_context(tc.tile_pool(name="pv_psum", bufs=2, space="PSUM"))

        ident = const_pool.tile([P, P], bf16, tag="ident", name="ident")
        masks.make_identity(nc, ident[:])
        identf = const_pool.tile([P, P], f32, tag="identf", name="identf")
        masks.make_identity(nc, identf[:])
        ones_row = const_pool.tile([1, P], bf16, tag="ones_row", name="ones_row")
        nc.vector.memset(ones_row[:], 1.0)
        ones_col = const_pool.tile([P, 1], bf16, tag="ones_col", name="ones_col")
        nc.vector.memset(ones_col[:], 1.0)
        # fp8 ones pair for the DoubleRow l-ride (padded so pair-step is 16B)
        ones8 = const_pool.tile([P, 32], fp8, tag="ones8", name="ones8")
        nc.vector.memset(ones8[:], 1.0)
        ones8v = ones8[:].rearrange("p (two x) -> p two x", two=2)[:, :, 0:1]
        # 1 - I mask to zero the diagonal of E
        maskoff = const_pool.tile([P, P], bf16, tag="maskoff", name="maskoff")
        nc.vector.tensor_scalar_mul(maskoff[:], ident[:], -1.0)
        nc.scalar.add(maskoff[:], maskoff[:], 1.0)

        xf_all = persist.tile([P, NI, D], f32, tag="xf", name="xf_all")
        X_bf = persist.tile([P, NI * D], bf16, tag="xbf", name="xbf")
        X8 = persist.tile([P, NI * D], fp8, tag="x8", name="x8")
        Xt8 = persist.tile([P, NK * S], fp8, tag="xt8", name="xt8")
        sq = persist.tile([P, NK * S], bf16, tag="sq", name="sq")
        negc = persist.tile([1, S], f32, tag="negc", name="negc")
        negc_h = persist.tile([1, S], bf16, tag="negch", name="negch")
        negc_l = persist.tile([1, S], bf16, tag="negcl", name="negcl")
        negc_full = persist.tile([P, S], f32, tag="negc_full", name="negc_full")
        E8 = persist.tile([P, NI * S], fp8, tag="et", name="et")

        Xt8_3 = Xt8[:].rearrange("p (k s) -> p k s", k=NK)
        Xt8_4 = Xt8[:].rearrange("p (k2 two s) -> p k2 two s", k2=NK // 2, two=2)
        sq3 = sq[:].rearrange("p (k s) -> p k s", k=NK)
        E8_4 = E8[:].rearrange("p (m two s) -> p m two s", m=NPAIR, two=2)
        X8_4 = X8[:].rearrange("p (m two d) -> p m two d", m=NPAIR, two=2)

        # ---- emit helpers ----
        def emit_warmup(n):
            # dummy matmuls: keep the PE busy so the HAM clock gate opens
            # (K=8/8, 2.4 GHz) before the real matmuls arrive. Cycles the qk
            # pool's two 2-bank tiles so back-to-back issue never serializes.
            for r in range(n):
                wm = qk_psum.tile([P, 2, JC], f32, tag="qk", name=f"warm{r}")
                nc.tensor.matmul(wm[:, 0, 0:P], lhsT=ident[:], rhs=ident[:], skip_group_check=True)

        def emit_load_tile(i):
            dcols = slice(i * D, (i + 1) * D)
            nc.sync.dma_start(xf_all[:, i], inp[i * P : (i + 1) * P, :])
            # PE transpose straight from f32 (transpose_mode is full rate for
            # fp32 on trn2) -- no bf16 staging cast on the scalar engine.
            pt = tr_psum.tile([P, NK, P], f32, tag="pt", name=f"ptx{i}")
            for k in range(NK):
                nc.tensor.matmul(
                    pt[:, k],
                    lhsT=xf_all[:, i, k * P : (k + 1) * P],
                    rhs=identf[:],
                    is_transpose=True,
                    skip_group_check=True,
                )
            nc.vector.tensor_copy(Xt8_3[:, :, i * P : (i + 1) * P], pt[:])
            # natural-layout fp8 copy for the PV moving operand; alternate
            # engines so neither paces the load pipeline
            eng = nc.vector if i % 2 == 0 else nc.gpsimd
            eng.tensor_copy(X8[:, dcols], xf_all[:, i])

        def emit_sq_muls(ic):
            ccols = slice(ic * JC, (ic + 1) * JC)
            for k in range(NK):
                eng = nc.gpsimd if k % 2 == 0 else nc.vector
                eng.tensor_mul(
                    sq3[:, k, ccols], Xt8_3[:, k, ccols], Xt8_3[:, k, ccols]
                )

        def emit_negc(ic, from_qk_pool=False):
            # c[s] = sum_d X[s, d]^2 for chunk ic's columns, negated and
            # broadcast to all partitions of negc_full (f32 via bf16 hi+lo
            # broadcast matmuls to keep full precision on the PE).
            # NOTE: the pc matmul waits on the sq muls; since the PE queue is
            # strictly in-order, this must only be emitted where that wait is
            # guaranteed resolved (else it head-of-line blocks the PE).
            ccols = slice(ic * JC, (ic + 1) * JC)
            t = qk_psum.tile([P, 2, JC], f32, tag="qk", name=f"negc{ic}")
            pc_ap = t[0:1, 0, :]
            pb_ap = t[:, 1, :]
            for k in range(NK):
                nc.tensor.matmul(
                    pc_ap,
                    lhsT=ones_col[:],
                    rhs=sq3[:, k, ccols],
                    start=(k == 0),
                    stop=(k == NK - 1),
                )
            nc.vector.tensor_scalar_mul(negc[:, ccols], pc_ap, -1.0)
            nc.vector.tensor_copy(negc_h[:, ccols], negc[:, ccols])
            nc.vector.tensor_sub(negc_l[:, ccols], negc[:, ccols], negc_h[:, ccols])
            nc.tensor.matmul(pb_ap, lhsT=ones_row[:], rhs=negc_h[:, ccols], start=True, stop=False)
            nc.tensor.matmul(pb_ap, lhsT=ones_row[:], rhs=negc_l[:, ccols], start=False, stop=True)
            # psum->sbuf copy on the scalar engine: keeps the DVE free at the
            # ramp tail so the main loop's first rides aren't stalled
            nc.scalar.copy(negc_full[:, ccols], pb_ap)

        def emit_qk_pair(ic, jp):
            # two key blocks (2jp, 2jp+1) x one query chunk into a 2-bank
            # psum tile; one broadcast -c add, ONE exp over both tiles.
            ccols = slice(ic * JC, (ic + 1) * JC)
            ps = qk_psum.tile([P, 2, JC], f32, tag="qk", name=f"qk{ic}_{jp}")
            for t in range(2):
                jt = 2 * jp + t
                for k2 in range(NK // 2):
                    nc.tensor.matmul(
                        ps[:, t],
                        lhsT=Xt8_4[:, k2, :, jt * P : (jt + 1) * P],
                        rhs=Xt8_4[:, k2, :, ccols],
                        perf_mode=DR,
                        start=(k2 == 0),
                        stop=(k2 == NK // 2 - 1),
                    )
            nbc = negc_full[:, ccols][:, None, :].broadcast_to([P, 2, JC])
            nc.vector.tensor_add(ps[:], ps[:], nbc)
            nc.scalar.activation(E8_4[:, jp, :, ccols], ps[:], AF.Exp)
            # diagonal blocks of this chunk: zero the diag (E = I + E_off).
            # On gpsimd: keeps the DVE's add->exp drain latency jitter-free.
            for t in range(2):
                jt = 2 * jp + t
                if ic * NSUB <= jt < (ic + 1) * NSUB:
                    dslice = slice(jt * S + jt * P, jt * S + (jt + 1) * P)
                    nc.gpsimd.tensor_mul(E8[:, dslice], E8[:, dslice], maskoff[:])

        # ---- startup: stream tiles in, interleaving chunk 0's QK pairs and
        # every chunk's sq/negc as soon as their operand tiles have landed,
        # so the PE and the drain engines ramp while the DMA streams. ----
        # X_bf (bf16, for the identity-ride matmuls only) is cast on the
        # scalar engine at LOW priority: each cast is emitted after an exp,
        # so the qk drain chain always runs ahead of the casts.
        cast_next = [0]

        def emit_xbf_cast(n=1):
            for _ in range(n):
                i = cast_next[0]
                if i >= NI:
                    return
                cast_next[0] += 1
                nc.scalar.copy(X_bf[:, i * D : (i + 1) * D], xf_all[:, i])

        emit_warmup(40)
        for i in range(NI):
            emit_load_tile(i)
            if i == 3:
                emit_sq_muls(0)
                emit_negc(0)
            elif i == 4:
                emit_qk_pair(0, 0)
                emit_xbf_cast()
                emit_qk_pair(0, 1)
                emit_xbf_cast()
            elif i >= 5 and i % 2 == 1:
                emit_qk_pair(0, (i - 1) // 2)
                emit_xbf_cast()
                if i == 7:
                    emit_sq_muls(1)
                elif i == 9:
                    emit_negc(1)
                elif i == 11:
                    emit_sq_muls(2)
                elif i == 13:
                    emit_negc(2)
                elif i == 15:
                    emit_sq_muls(3)
        emit_xbf_cast(3)

        def emit_pv_start(i, po, pl):
            # identity ride: po += I @ X_bf[i]  (adds x_i at bf16 precision),
            # pl += I @ 1 (adds the diagonal's weight of exactly 1)
            nc.tensor.matmul(
                po[:],
                lhsT=ident[:],
                rhs=X_bf[:, i * D : (i + 1) * D],
                start=True,
                stop=False,
            )
            nc.tensor.matmul(
                pl[:], lhsT=ident[:], rhs=ones_col[:], start=True, stop=False
            )

        def emit_pv_pair(i, po, pl, m):
            lhsT = E8_4[:, m, :, i * P : (i + 1) * P]
            nc.tensor.matmul(
                po[:],
                lhsT=lhsT,
                rhs=X8_4[:, m],
                perf_mode=DR,
                start=False,
                stop=(m == NPAIR - 1),
            )
            nc.tensor.matmul(
                pl[:],
                lhsT=lhsT,
                rhs=ones8v,
                perf_mode=DR,
                start=False,
                stop=(m == NPAIR - 1),
            )

        # The osb scale-activation runs on the scalar engine, which is also
        # the qk drain path (exp). Emission order == FIFO order per engine,
        # so osb is DEFERRED and flushed only after the next qk pair's exp:
        # the drains always lead the scalar queue.
        pending_osb = []

        def emit_pv_end(i, po, pl):
            linv = stat_pool.tile([P, 1], f32, tag="linv", name=f"linv{i}")
            nc.vector.reciprocal(linv[:], pl[:])
            pending_osb.append((i, po, linv))

        def flush_osb():
            while pending_osb:
                i, po, linv = pending_osb.pop(0)
                osb = osb_pool.tile([P, D], f32, tag="osb", name=f"osb{i}")
                if i >= NI - 2:
                    # last blocks: halve the drain chain so the final DMA
                    # overlaps the second half's scale activation
                    h = D // 2
                    nc.scalar.activation(osb[:, :h], po[:, :h], AF.Copy, scale=linv[:])
                    nc.sync.dma_start(out[i * P : (i + 1) * P, :h], osb[:, :h])
                    nc.scalar.activation(osb[:, h:], po[:, h:], AF.Copy, scale=linv[:])
                    nc.sync.dma_start(out[i * P : (i + 1) * P, h:], osb[:, h:])
                else:
                    nc.scalar.activation(osb[:], po[:], AF.Copy, scale=linv[:])
                    nc.sync.dma_start(out[i * P : (i + 1) * P, :], osb[:])


        # Ramp/main boundary: the last ramp QK pairs' drains can't complete
        # before ~22us (DMA-floor bound), so the first main QK pairs stall on
        # psum banks. Fill that window with warm dummy matmuls so the PE
        # clock gate never re-throttles (idle -> half clock for ~14us).
        # two l accumulators share one tr psum bank (halves, alternating per
        # i-block). Allocated after the last transpose tile, whose DVE copy
        # completes early in the ramp tail -- no late WAR.
        l2 = tr_psum.tile([P, 4], f32, tag="pt", name="l2")

        wmain = pv_psum.tile([P, D], f32, tag="pv", name="wmain")
        for r in range(36):
            nc.tensor.matmul(
                wmain[:, 0:P], lhsT=ident[:], rhs=ident[:], skip_group_check=True
            )

        # Main loop: PV steps LEAD the QK pair stream in program order. The
        # PE queue is strictly in-order, so a QK pair waiting on its psum
        # bank's add+exp drain must have runnable PV work emitted BEFORE it,
        # not after. PV leads by TWO steps at every chunk boundary (the QK
        # bank drains of the previous chunk's tail are still in flight
        # there), and each block consumes its key pairs in order
        # [2,3,4,5],[0,1,6,7] so the freshest pair (7) is needed last.
        sched = []
        for c in range(NC):
            sched.append(("pv", c, 0))
            sched.append(("pv", c, 1))
            for jp in range(NPAIR):
                if c + 1 < NC:
                    sched.append(("qk", c + 1, jp))
                if 2 + jp < NPAIR:
                    sched.append(("pv", c, 2 + jp))

        PAIR_ORDER = [[2, 3, 4, 5], [0, 1, 6, 7]]
        po = pl = None
        for kind, a, b in sched:
            if kind == "qk":
                emit_qk_pair(a, b)
                flush_osb()
                emit_xbf_cast()
                if (a, b) == (1, 2):
                    # chunk 3's negc: its sq muls (emitted at load 15) have
                    # drained by the time the PE reaches this point, and
                    # QK(3) only consumes it ~15us later.
                    emit_negc(3, from_qk_pool=True)
            else:
                c, s = a, b
                i = c * NSUB + s // 2
                if s % 2 == 0:
                    po = pv_psum.tile([P, D], f32, tag="pv", name=f"pv{i}")
                    pl = l2[:, (i % 4) : (i % 4) + 1]
                    emit_pv_start(i, po, pl)
                for m in PAIR_ORDER[s % 2]:
                    emit_pv_pair(i, po, pl, m)
                if s % 2 == 1:
                    emit_pv_end(i, po, pl)
                    if c == NC - 1:
                        # no qk pairs left to defer behind
                        flush_osb()

    nc.compile()
    return nc


def _maybe_install_trace_hook():
    """Install the NTFF profile hook (test/profiling only; optional)."""
    import sys
    import types

    try:
        from antenv.axon_hooks import get_axon_ntff_profile_hook  # noqa: F401

        return  # already available
    except ImportError:
        pass
    try:
        mod = types.ModuleType("antenv.axon_hooks")
        _hook = [None]
        mod.set_axon_ntff_profile_hook = lambda h: _hook.__setitem__(0, h)
        mod.get_axon_ntff_profile_hook = lambda: _hook[0]
        sys.modules["antenv.axon_hooks"] = mod
        import antenv

        antenv.axon_hooks = mod
        from trn_agent_boot.trn_boot import _ntff_profile_via_ctypes

        mod.set_axon_ntff_profile_hook(
            _ntff_profile_via_ctypes("/opt/axon/libaxon_pjrt.so")
        )
    except Exception:
        pass


def kernel(inputs: np.ndarray) -> np.ndarray:
    from concourse.bass_utils import run_bass_kernel_spmd

    x = np.ascontiguousarray(np.asarray(inputs, dtype=np.float32))
    assert x.shape == (B, S, D), f"unexpected input shape {x.shape}"

    if "nc" not in _CACHE:
        _CACHE["nc"] = _build_nc()
    nc = _CACHE["nc"]

    trace = bool(int(os.environ.get("ATT_KERNEL_TRACE", "0")))
    if trace:
        _maybe_install_trace_hook()

    in_maps = [{"inputs": x[b]} for b in range(B)]
    res = run_bass_kernel_spmd(nc, in_maps, core_ids=list(range(B)), trace=trace)
    kernel.last_exec_time_ns = res.exec_time_ns
    return np.stack([res.results[b]["out"] for b in range(B)], axis=0)


kernel.last_exec_time_ns = None
